# revision 9
# baseline (speedup 1.0000x reference)
"""CommNet actor kernel for Trainium2 (Bass/Tile), 8-core data-parallel.

Math (per sample, A=32 agents, D=128 obs, H=64 hidden, NA=16 actions):
    h   = tanh(obs @ enc_w + enc_b)
    2 rounds of:  messages = h @ comm_w + comm_b
                  received = (sum_agents(messages) - messages) / (A-1)
                  h = tanh([h, received] @ upd_w + upd_b)
    out = tanh(h @ dec_w1 + dec_b1) @ dec_w2 + dec_b2

The round is folded on the host into  h' = tanh(h @ W1 + s @ W2 + b)  where
s = sum_agents(h), W1 = U_top - comm_w @ U_bot / (A-1), W2 = comm_w @ U_bot / (A-1),
b = comm_b @ U_bot + upd_b   (U_top/U_bot = upd_w[:H], upd_w[H:]).

Device layout: feature-major activations [feat, tok]. Each "unit" is 2048
tokens; the first 1024 tokens (T0) live on SBUF/PSUM partitions 0:64, the
second 1024 (T1) on partitions 64:128 (PE row/col-group packing -> the two
halves' matmuls run concurrently on disjoint 64x64 quadrants of the PE array,
and tanh/reduce ops process both halves in single [128, 1024] instructions).

obs is pre-transposed on the host into the exact feature-major DMA layout, so
all HBM traffic is contiguous. K=64 weights are stacked [W; W] on 128
partitions so each half reads its own rows.
"""

import numpy as np
from contextlib import ExitStack

import concourse.bass as bass
import concourse.bacc as bacc
import concourse.tile as tile
from concourse import mybir
from concourse.bass_utils import run_bass_kernel_spmd

# Problem constants
B, A, D, H, NA = 16384, 32, 128, 64, 16
R = 2
NCORES = 8
S_CORE = B // NCORES          # 2048 samples per core
TOK = S_CORE * A              # 65536 tokens per core
HALF_TOK = 1024               # tokens per half-unit (32 samples)
UNIT_TOK = 2 * HALF_TOK       # 2048 tokens per unit
NU = TOK // UNIT_TOK          # 32 units per core
SAMP_HALF = HALF_TOK // A     # 32 samples per half-unit
CHUNK = 128                   # dec2 token chunk (output partition dim)
NCHUNK = HALF_TOK // CHUNK    # 8 chunks per half-unit
FP = mybir.dt.float32
TANH = mybir.ActivationFunctionType.Tanh

# wpack column layout
_C_ENC = 0            # enc_w        [128, 64]
_C_W1 = (64, 192)     # W1 stacked   [128, 64] per round
_C_W2 = (128, 256)    # W2 stacked   [128, 64] per round
_C_D1 = 320           # dec_w1 stacked [128, 64]
_C_D2 = 384           # dec_w2 stacked [128, 16]
_C_BE = 400           # bias cols: enc, r0, r1, dec1
NW = 404


def build_body(ctx, tc, obs_t, wpack, out, n_units):
    nc = tc.nc
    wpool = ctx.enter_context(tc.tile_pool(name="w", bufs=1))
    obs_pool = ctx.enter_context(tc.tile_pool(name="obs", bufs=6))
    h_pool = ctx.enter_context(tc.tile_pool(name="h", bufs=8))
    s_pool = ctx.enter_context(tc.tile_pool(name="s", bufs=4))
    osb_pool = ctx.enter_context(tc.tile_pool(name="osb", bufs=4))
    ps_pool = ctx.enter_context(tc.tile_pool(name="ps", bufs=3, space="PSUM"))
    po_pool = ctx.enter_context(tc.tile_pool(name="po", bufs=2, space="PSUM"))

    w = wpool.tile([D, NW], FP)
    nc.sync.dma_start(out=w[:], in_=wpack)

    w_enc = w[:, 0:64]
    w1 = [w[:, _C_W1[r] : _C_W1[r] + 64] for r in range(R)]
    w2 = [w[:, _C_W2[r] : _C_W2[r] + 64] for r in range(R)]
    w_d1 = w[:, _C_D1 : _C_D1 + 64]
    w_d2 = w[:, _C_D2 : _C_D2 + NA]
    b_enc = w[:, _C_BE : _C_BE + 1]
    b_r = [w[:, _C_BE + 1 + r : _C_BE + 2 + r] for r in range(R)]
    b_d1 = w[:, _C_BE + 3 : _C_BE + 4]

    c0 = slice(0, 512)
    c1 = slice(512, 1024)
    lo = slice(0, 64)
    hi = slice(64, 128)

    # out DRAM layout is the DMA walk order itself: [u, p, hh, c, e] (fully
    # contiguous stores; host transposes back to token order afterwards)
    out_v = out.rearrange("(u p hh c) e -> u p hh c e", p=CHUNK, hh=2, c=NCHUNK)

    for u in range(n_units):
        obs0 = obs_pool.tile([D, HALF_TOK], FP, tag="obs")
        obs1 = obs_pool.tile([D, HALF_TOK], FP, tag="obs")
        nc.sync.dma_start(out=obs0[:], in_=obs_t[u, 0])
        nc.sync.dma_start(out=obs1[:], in_=obs_t[u, 1])

        # encoder: K=128 col-split; crosswise bank order for T0/T1 concurrency
        ps_e = ps_pool.tile([128, HALF_TOK], FP, tag="ps")
        nc.tensor.matmul(ps_e[lo, c0], lhsT=w_enc, rhs=obs0[:, c0], skip_group_check=True)
        nc.tensor.matmul(ps_e[hi, c1], lhsT=w_enc, rhs=obs1[:, c1], skip_group_check=True)
        nc.tensor.matmul(ps_e[lo, c1], lhsT=w_enc, rhs=obs0[:, c1], skip_group_check=True)
        nc.tensor.matmul(ps_e[hi, c0], lhsT=w_enc, rhs=obs1[:, c0], skip_group_check=True)

        h = h_pool.tile([128, HALF_TOK], FP, tag="h")
        nc.scalar.activation(h[:], ps_e[:], TANH, bias=b_enc)

        for r in range(R):
            s = s_pool.tile([128, SAMP_HALF], FP, tag="s")
            nc.vector.reduce_sum(
                out=s[:],
                in_=h.rearrange("p (g a) -> p g a", a=A),
                axis=mybir.AxisListType.X,
            )
            ns = SAMP_HALF // 2  # samples per 512-token column block
            sb = [
                s[p, b * ns : (b + 1) * ns].unsqueeze(2).broadcast_to([64, ns, A])
                for p, b in ((lo, 0), (lo, 1), (hi, 0), (hi, 1))
            ]
            ps_r = ps_pool.tile([128, HALF_TOK], FP, tag="ps")
            nc.tensor.matmul(ps_r[lo, c0], lhsT=w1[r][lo], rhs=h[lo, c0], start=True, stop=False, skip_group_check=True)
            nc.tensor.matmul(ps_r[hi, c1], lhsT=w1[r][hi], rhs=h[hi, c1], start=True, stop=False, skip_group_check=True)
            nc.tensor.matmul(ps_r[lo, c1], lhsT=w1[r][lo], rhs=h[lo, c1], start=True, stop=False, skip_group_check=True)
            nc.tensor.matmul(ps_r[hi, c0], lhsT=w1[r][hi], rhs=h[hi, c0], start=True, stop=False, skip_group_check=True)
            nc.tensor.matmul(ps_r[lo, c0], lhsT=w2[r][lo], rhs=sb[0], start=False, stop=True, skip_group_check=True)
            nc.tensor.matmul(ps_r[hi, c1], lhsT=w2[r][hi], rhs=sb[3], start=False, stop=True, skip_group_check=True)
            nc.tensor.matmul(ps_r[lo, c1], lhsT=w2[r][lo], rhs=sb[1], start=False, stop=True, skip_group_check=True)
            nc.tensor.matmul(ps_r[hi, c0], lhsT=w2[r][hi], rhs=sb[2], start=False, stop=True, skip_group_check=True)

            h = h_pool.tile([128, HALF_TOK], FP, tag="h")
            nc.scalar.activation(h[:], ps_r[:], TANH, bias=b_r[r])

        # dec1
        ps_d = ps_pool.tile([128, HALF_TOK], FP, tag="ps")
        nc.tensor.matmul(ps_d[lo, c0], lhsT=w_d1[lo], rhs=h[lo, c0], skip_group_check=True)
        nc.tensor.matmul(ps_d[hi, c1], lhsT=w_d1[hi], rhs=h[hi, c1], skip_group_check=True)
        nc.tensor.matmul(ps_d[lo, c1], lhsT=w_d1[lo], rhs=h[lo, c1], skip_group_check=True)
        nc.tensor.matmul(ps_d[hi, c0], lhsT=w_d1[hi], rhs=h[hi, c0], skip_group_check=True)
        pre = h_pool.tile([128, HALF_TOK], FP, tag="h")
        nc.scalar.activation(pre[:], ps_d[:], TANH, bias=b_d1)

        # dec2: flipped orientation -> token-major [128 tok, 16] outputs
        po0 = po_pool.tile([128, NCHUNK * NA], FP, tag="po")
        po1 = po_pool.tile([128, NCHUNK * NA], FP, tag="po")
        for c in range(NCHUNK):
            cs = slice(c * CHUNK, (c + 1) * CHUNK)
            os_ = slice(c * NA, (c + 1) * NA)
            nc.tensor.matmul(po0[:, os_], lhsT=pre[lo, cs], rhs=w_d2[lo], skip_group_check=True)
            nc.tensor.matmul(po1[:, os_], lhsT=pre[hi, cs], rhs=w_d2[hi], skip_group_check=True)

        osb = osb_pool.tile([128, 2 * NCHUNK * NA], FP, tag="osb")
        nc.vector.tensor_copy(osb[:, 0 : NCHUNK * NA], po0[:])
        nc.vector.tensor_copy(osb[:, NCHUNK * NA :], po1[:])

        nc.sync.dma_start(
            out=out_v[u], in_=osb.rearrange("p (hh c e) -> p hh c e", hh=2, e=NA)
        )


def build_nc(n_units=NU):
    nc = bacc.Bacc(None, target_bir_lowering=False, debug=False)
    obs_t = nc.declare_dram_parameter(
        "obs_t", [n_units, 2, D, HALF_TOK], FP, isOutput=False
    )
    wpack = nc.declare_dram_parameter("wpack", [D, NW], FP, isOutput=False)
    out = nc.declare_dram_parameter(
        "out", [n_units * UNIT_TOK, NA], FP, isOutput=True
    )
    with tile.TileContext(nc) as tc:
        with ExitStack() as ctx:
            build_body(ctx, tc, obs_t[:], wpack[:], out[:], n_units)
    nc.compile()
    return nc


def fold_weights(enc_w, enc_b, comm_w, comm_b, upd_w, upd_b, dec_w1, dec_b1, dec_w2):
    """Host-side algebraic fold + packing into the wpack tensor (float64 math)."""
    f8 = np.float64
    denom = f8(max(A - 1, 1))
    wpack = np.zeros((D, NW), np.float32)
    wpack[:, 0:64] = np.asarray(enc_w, np.float32)
    for r in range(R):
        C = np.asarray(comm_w[r], f8)
        Ut = np.asarray(upd_w[r][:H], f8)
        Ub = np.asarray(upd_w[r][H:], f8)
        G = C @ Ub / denom
        W1 = (Ut - G).astype(np.float32)
        W2 = G.astype(np.float32)
        br = (np.asarray(comm_b[r], f8) @ Ub + np.asarray(upd_b[r], f8)).astype(
            np.float32
        )
        wpack[0:64, _C_W1[r] : _C_W1[r] + 64] = W1
        wpack[64:128, _C_W1[r] : _C_W1[r] + 64] = W1
        wpack[0:64, _C_W2[r] : _C_W2[r] + 64] = W2
        wpack[64:128, _C_W2[r] : _C_W2[r] + 64] = W2
        wpack[0:64, _C_BE + 1 + r] = br
        wpack[64:128, _C_BE + 1 + r] = br
    d1 = np.asarray(dec_w1, np.float32)
    wpack[0:64, _C_D1 : _C_D1 + 64] = d1
    wpack[64:128, _C_D1 : _C_D1 + 64] = d1
    d2 = np.asarray(dec_w2, np.float32)
    wpack[0:64, _C_D2 : _C_D2 + NA] = d2
    wpack[64:128, _C_D2 : _C_D2 + NA] = d2
    be = np.asarray(enc_b, np.float32)
    wpack[0:64, _C_BE] = be
    wpack[64:128, _C_BE] = be
    bd1 = np.asarray(dec_b1, np.float32)
    wpack[0:64, _C_BE + 3] = bd1
    wpack[64:128, _C_BE + 3] = bd1
    return wpack


def prep_obs(obs):
    """[B, A, D] -> [NCORES, NU, 2, D, HALF_TOK] feature-major contiguous."""
    obs5 = np.asarray(obs, np.float32).reshape(NCORES, NU, 2, HALF_TOK, D)
    return np.ascontiguousarray(obs5.transpose(0, 1, 2, 4, 3))


_NC_CACHE = {}


def _get_nc(n_units=NU):
    if n_units not in _NC_CACHE:
        _NC_CACHE[n_units] = build_nc(n_units)
    return _NC_CACHE[n_units]


def kernel(
    obs,
    enc_w,
    enc_b,
    comm_w,
    comm_b,
    upd_w,
    upd_b,
    dec_w1,
    dec_b1,
    dec_w2,
    dec_b2,
    _trace=False,
    _trace_kwargs=None,
):
    wpack = fold_weights(
        enc_w, enc_b, comm_w, comm_b, upd_w, upd_b, dec_w1, dec_b1, dec_w2
    )
    obs_t = prep_obs(obs)
    nc = _get_nc()
    in_maps = [{"obs_t": obs_t[i], "wpack": wpack} for i in range(NCORES)]
    res = run_bass_kernel_spmd(
        nc,
        in_maps,
        core_ids=list(range(NCORES)),
        trace=_trace,
        **(_trace_kwargs or {}),
    )
    outs = np.stack([res.results[i]["out"] for i in range(NCORES)])
    # device order is [u, p, hh, c, e]; token t = u*2048 + hh*1024 + c*128 + p
    outs = outs.reshape(NCORES, NU, CHUNK, 2, NCHUNK, NA)
    outs = outs.transpose(0, 1, 3, 4, 2, 5)  # -> [core, u, hh, c, p, e]
    logits = outs.reshape(B, A, NA) + np.asarray(dec_b2, np.float32)[None, None, :]
    if _trace:
        return logits.astype(np.float32), res
    return logits.astype(np.float32)


# revision 13
# speedup vs baseline: 1.5028x; 1.5028x over previous
"""CommNet actor kernel for Trainium2 (Bass/Tile), 8-core data-parallel.

Math (per sample, A=32 agents, D=128 obs, H=64 hidden, NA=16 actions):
    h   = tanh(obs @ enc_w + enc_b)
    2 rounds of:  messages = h @ comm_w + comm_b
                  received = (sum_agents(messages) - messages) / (A-1)
                  h = tanh([h, received] @ upd_w + upd_b)
    out = tanh(h @ dec_w1 + dec_b1) @ dec_w2 + dec_b2

The round is folded on the host into  h' = tanh(h @ W1 + s @ W2 + b)  where
s = sum_agents(h), W1 = U_top - comm_w @ U_bot / (A-1), W2 = comm_w @ U_bot / (A-1),
b = comm_b @ U_bot + upd_b   (U_top/U_bot = upd_w[:H], upd_w[H:]).

Device layout: feature-major activations [feat, tok]. Each "unit" is 2048
tokens; the first 1024 tokens (T0) live on SBUF/PSUM partitions 0:64, the
second 1024 (T1) on partitions 64:128. All matmuls run in float32r (single-
pass PE mode; plain fp32 costs 2 half-speed passes). f32r only supports
tile_position (0,0), so the two halves are computed with block-diagonal
weights kron(I2, W) in one full-array matmul; the encoder stacks halves via
a zero-padded lhsT accumulation pair. tanh/reduce then process both halves
in single [128, 1024] instructions (full 128-lane utilization).

obs is pre-transposed on the host into the exact feature-major DMA layout, so
all HBM traffic is contiguous; the output is stored in DMA walk order and
transposed back on the host.
"""

import numpy as np
from contextlib import ExitStack

import concourse.bass as bass
import concourse.bacc as bacc
import concourse.tile as tile
from concourse import mybir
from concourse.bass_utils import run_bass_kernel_spmd

# Problem constants
B, A, D, H, NA = 16384, 32, 128, 64, 16
R = 2
NCORES = 8
S_CORE = B // NCORES          # 2048 samples per core
TOK = S_CORE * A              # 65536 tokens per core
HALF_TOK = 1024               # tokens per half-unit (32 samples)
UNIT_TOK = 2 * HALF_TOK       # 2048 tokens per unit
NU = TOK // UNIT_TOK          # 32 units per core
SAMP_HALF = HALF_TOK // A     # 32 samples per half-unit
CHUNK = 128                   # dec2 token chunk (output partition dim)
NCHUNK = HALF_TOK // CHUNK    # 8 chunks per half-unit
FP = mybir.dt.float32
FR = mybir.dt.float32r  # single-pass PE mode (fp32 is 2 half-speed passes)
TANH = mybir.ActivationFunctionType.Tanh


def _f(ap):
    return ap.bitcast(FP)


# wpack column layout
_C_ENC = 0              # enc_w                 [128, 64]   (T0 encoder)
_C_ENCP = 64            # [0 | enc_w]           [128, 128]  (T1 encoder, zero-pad)
_C_W1 = (192, 448)      # kron(I2, W1_r)        [128, 128] per round
_C_W2 = (320, 576)      # kron(I2, W2_r)        [128, 128] per round
_C_D1 = 704             # kron(I2, dec_w1)      [128, 128]
_C_D2 = 832             # kron(I2, dec_w2)      [128, 32]
_C_BE = 864             # bias cols: enc, r0, r1, dec1 (each stacked [b; b])
NW = 868


def build_body(ctx, tc, obs_t, wpack, out, n_units):
    nc = tc.nc
    wpool = ctx.enter_context(tc.tile_pool(name="w", bufs=1))
    obs_pool = ctx.enter_context(tc.tile_pool(name="obs", bufs=6))
    h_pool = ctx.enter_context(tc.tile_pool(name="h", bufs=8))
    s_pool = ctx.enter_context(tc.tile_pool(name="s", bufs=4))
    osb_pool = ctx.enter_context(tc.tile_pool(name="osb", bufs=4))
    ps_pool = ctx.enter_context(tc.tile_pool(name="ps", bufs=3, space="PSUM"))
    po_pool = ctx.enter_context(tc.tile_pool(name="po", bufs=2, space="PSUM"))

    w = wpool.tile([D, NW], FR)
    nc.sync.dma_start(out=w[:], in_=wpack)

    w_enc = w[:, _C_ENC : _C_ENC + 64]
    w_encp = w[:, _C_ENCP : _C_ENCP + 128]
    w1 = [w[:, _C_W1[r] : _C_W1[r] + 128] for r in range(R)]
    w2 = [w[:, _C_W2[r] : _C_W2[r] + 128] for r in range(R)]
    w_d1 = w[:, _C_D1 : _C_D1 + 128]
    w_d2 = w[:, _C_D2 : _C_D2 + 32]
    b_enc = _f(w[:, _C_BE : _C_BE + 1])
    b_r = [_f(w[:, _C_BE + 1 + r : _C_BE + 2 + r]) for r in range(R)]
    b_d1 = _f(w[:, _C_BE + 3 : _C_BE + 4])

    c0 = slice(0, 512)
    c1 = slice(512, 1024)
    lo = slice(0, 64)

    # out DRAM layout is the DMA walk order itself: [u, p, c, hh, e] (fully
    # contiguous stores; host transposes back to token order afterwards)
    out_v = out.rearrange("(u p c hh) e -> u p c hh e", p=CHUNK, c=NCHUNK, hh=2)

    for u in range(n_units):
        obs0 = obs_pool.tile([D, HALF_TOK], FR, tag="obs")
        obs1 = obs_pool.tile([D, HALF_TOK], FR, tag="obs")
        nc.sync.dma_start(out=obs0[:], in_=obs_t[u, 0])
        nc.sync.dma_start(out=obs1[:], in_=obs_t[u, 1])

        # encoder: T1 via zero-padded M=128 lhsT (start), T0 accumulates M=64
        ps_e = ps_pool.tile([128, HALF_TOK], FP, tag="ps")
        for cs in (c0, c1):
            nc.tensor.matmul(ps_e[:, cs], lhsT=w_encp, rhs=obs1[:, cs],
                             start=True, stop=False, skip_group_check=True)
            nc.tensor.matmul(ps_e[lo, cs], lhsT=w_enc, rhs=obs0[:, cs],
                             start=False, stop=True, skip_group_check=True)

        h = h_pool.tile([128, HALF_TOK], FR, tag="h")
        nc.scalar.activation(h[:], ps_e[:], TANH, bias=b_enc)

        for r in range(R):
            s = s_pool.tile([128, SAMP_HALF], FR, tag="s")
            with nc.allow_low_precision(
                reason="float32r is 4-byte fp32; PE needs f32r-typed operands"
            ):
                nc.vector.reduce_sum(
                    out=s[:],
                    in_=h.rearrange("p (g a) -> p g a", a=A),
                    axis=mybir.AxisListType.X,
                )
            ns = SAMP_HALF // 2  # samples per 512-token column block
            ps_r = ps_pool.tile([128, HALF_TOK], FP, tag="ps")
            for b, cs in ((0, c0), (1, c1)):
                sb = s[:, b * ns : (b + 1) * ns].unsqueeze(2).broadcast_to(
                    [128, ns, A]
                )
                nc.tensor.matmul(ps_r[:, cs], lhsT=w1[r], rhs=h[:, cs],
                                 start=True, stop=False, skip_group_check=True)
                nc.tensor.matmul(ps_r[:, cs], lhsT=w2[r], rhs=sb,
                                 start=False, stop=True, skip_group_check=True)

            h = h_pool.tile([128, HALF_TOK], FR, tag="h")
            nc.scalar.activation(h[:], ps_r[:], TANH, bias=b_r[r])

        # dec1
        ps_d = ps_pool.tile([128, HALF_TOK], FP, tag="ps")
        for cs in (c0, c1):
            nc.tensor.matmul(ps_d[:, cs], lhsT=w_d1, rhs=h[:, cs],
                             skip_group_check=True)
        pre = h_pool.tile([128, HALF_TOK], FR, tag="h")
        nc.scalar.activation(pre[:], ps_d[:], TANH, bias=b_d1)

        # dec2: flipped orientation; one K=128/M=128/N=32 matmul computes a
        # 128-token chunk for BOTH halves (cols 0:16 = T0, 16:32 = T1)
        po = po_pool.tile([128, NCHUNK * 32], FP, tag="po")
        for c in range(NCHUNK):
            nc.tensor.matmul(
                po[:, c * 32 : (c + 1) * 32],
                lhsT=pre[:, c * CHUNK : (c + 1) * CHUNK],
                rhs=w_d2,
                skip_group_check=True,
            )

        osb = osb_pool.tile([128, NCHUNK * 32], FP, tag="osb")
        nc.vector.tensor_copy(osb[:], po[:])

        nc.sync.dma_start(
            out=out_v[u], in_=osb.rearrange("p (c hh e) -> p c hh e", c=NCHUNK, e=NA)
        )


def build_nc(n_units=NU):
    nc = bacc.Bacc(None, target_bir_lowering=False, debug=False)
    obs_t = nc.declare_dram_parameter(
        "obs_t", [n_units, 2, D, HALF_TOK], FR, isOutput=False
    )
    wpack = nc.declare_dram_parameter("wpack", [D, NW], FR, isOutput=False)
    out = nc.declare_dram_parameter(
        "out", [n_units * UNIT_TOK, NA], FP, isOutput=True
    )
    with tile.TileContext(nc) as tc:
        with ExitStack() as ctx:
            build_body(ctx, tc, obs_t[:], wpack[:], out[:], n_units)
    nc.compile()
    return nc


def fold_weights(enc_w, enc_b, comm_w, comm_b, upd_w, upd_b, dec_w1, dec_b1, dec_w2):
    """Host-side algebraic fold + packing into the wpack tensor (float64 math)."""
    f8 = np.float64
    denom = f8(max(A - 1, 1))
    wpack = np.zeros((D, NW), np.float32)

    def bd(Wm):  # kron(I2, W) for [64, x] -> [128, 2x]
        Wm = np.asarray(Wm, np.float32)
        k, m = Wm.shape
        o = np.zeros((2 * k, 2 * m), np.float32)
        o[:k, :m] = Wm
        o[k:, m:] = Wm
        return o

    wpack[:, _C_ENC : _C_ENC + 64] = np.asarray(enc_w, np.float32)
    wpack[:, _C_ENCP + 64 : _C_ENCP + 128] = np.asarray(enc_w, np.float32)
    for r in range(R):
        C = np.asarray(comm_w[r], f8)
        Ut = np.asarray(upd_w[r][:H], f8)
        Ub = np.asarray(upd_w[r][H:], f8)
        G = C @ Ub / denom
        W1 = (Ut - G).astype(np.float32)
        W2 = G.astype(np.float32)
        br = (np.asarray(comm_b[r], f8) @ Ub + np.asarray(upd_b[r], f8)).astype(
            np.float32
        )
        wpack[:, _C_W1[r] : _C_W1[r] + 128] = bd(W1)
        wpack[:, _C_W2[r] : _C_W2[r] + 128] = bd(W2)
        wpack[0:64, _C_BE + 1 + r] = br
        wpack[64:128, _C_BE + 1 + r] = br
    wpack[:, _C_D1 : _C_D1 + 128] = bd(dec_w1)
    wpack[:, _C_D2 : _C_D2 + 32] = bd(dec_w2)
    be = np.asarray(enc_b, np.float32)
    wpack[0:64, _C_BE] = be
    wpack[64:128, _C_BE] = be
    bd1 = np.asarray(dec_b1, np.float32)
    wpack[0:64, _C_BE + 3] = bd1
    wpack[64:128, _C_BE + 3] = bd1
    return wpack


def prep_obs(obs):
    """[B, A, D] -> [NCORES, NU, 2, D, HALF_TOK] feature-major contiguous."""
    obs5 = np.asarray(obs, np.float32).reshape(NCORES, NU, 2, HALF_TOK, D)
    return np.ascontiguousarray(obs5.transpose(0, 1, 2, 4, 3))


_NC_CACHE = {}


def _get_nc(n_units=NU):
    if n_units not in _NC_CACHE:
        _NC_CACHE[n_units] = build_nc(n_units)
    return _NC_CACHE[n_units]


def kernel(
    obs,
    enc_w,
    enc_b,
    comm_w,
    comm_b,
    upd_w,
    upd_b,
    dec_w1,
    dec_b1,
    dec_w2,
    dec_b2,
    _trace=False,
    _trace_kwargs=None,
):
    wpack = fold_weights(
        enc_w, enc_b, comm_w, comm_b, upd_w, upd_b, dec_w1, dec_b1, dec_w2
    )
    obs_t = prep_obs(obs)
    nc = _get_nc()
    in_maps = [{"obs_t": obs_t[i], "wpack": wpack} for i in range(NCORES)]
    res = run_bass_kernel_spmd(
        nc,
        in_maps,
        core_ids=list(range(NCORES)),
        trace=_trace,
        **(_trace_kwargs or {}),
    )
    outs = np.stack([res.results[i]["out"] for i in range(NCORES)])
    # device order is [u, p, c, hh, e]; token t = u*2048 + hh*1024 + c*128 + p
    outs = outs.reshape(NCORES, NU, CHUNK, NCHUNK, 2, NA)
    outs = outs.transpose(0, 1, 4, 3, 2, 5)  # -> [core, u, hh, c, p, e]
    logits = outs.reshape(B, A, NA) + np.asarray(dec_b2, np.float32)[None, None, :]
    if _trace:
        return logits.astype(np.float32), res
    return logits.astype(np.float32)


# revision 15
# speedup vs baseline: 1.6026x; 1.0664x over previous
"""CommNet actor kernel for Trainium2 (Bass/Tile), 8-core data-parallel.

Math (per sample, A=32 agents, D=128 obs, H=64 hidden, NA=16 actions):
    h   = tanh(obs @ enc_w + enc_b)
    2 rounds of:  messages = h @ comm_w + comm_b
                  received = (sum_agents(messages) - messages) / (A-1)
                  h = tanh([h, received] @ upd_w + upd_b)
    out = tanh(h @ dec_w1 + dec_b1) @ dec_w2 + dec_b2

The round is folded on the host into  h' = tanh(h @ W1 + s @ W2 + b)  where
s = sum_agents(h), W1 = U_top - comm_w @ U_bot / (A-1), W2 = comm_w @ U_bot / (A-1),
b = comm_b @ U_bot + upd_b   (U_top/U_bot = upd_w[:H], upd_w[H:]).

Device layout: feature-major activations [feat, tok]. Each "unit" is 2048
tokens; the first 1024 tokens (T0) live on SBUF/PSUM partitions 0:64, the
second 1024 (T1) on partitions 64:128. All matmuls run in float32r (single-
pass PE mode; plain fp32 costs 2 half-speed passes). f32r only supports
tile_position (0,0), so the two halves are computed with block-diagonal
weights kron(I2, W) in one full-array matmul; the encoder stacks halves via
a zero-padded lhsT accumulation pair. tanh/reduce then process both halves
in single [128, 1024] instructions (full 128-lane utilization).

obs is pre-transposed on the host into the exact feature-major DMA layout, so
all HBM traffic is contiguous; the output is stored in DMA walk order and
transposed back on the host.
"""

import numpy as np
from contextlib import ExitStack

import concourse.bass as bass
import concourse.bacc as bacc
import concourse.tile as tile
from concourse import mybir
from concourse.bass_utils import run_bass_kernel_spmd

# Problem constants
B, A, D, H, NA = 16384, 32, 128, 64, 16
R = 2
NCORES = 8
S_CORE = B // NCORES          # 2048 samples per core
TOK = S_CORE * A              # 65536 tokens per core
HALF_TOK = 1024               # tokens per half-unit (32 samples)
UNIT_TOK = 2 * HALF_TOK       # 2048 tokens per unit
NU = TOK // UNIT_TOK          # 32 units per core
SAMP_HALF = HALF_TOK // A     # 32 samples per half-unit
CHUNK = 128                   # dec2 token chunk (output partition dim)
NCHUNK = HALF_TOK // CHUNK    # 8 chunks per half-unit
FP = mybir.dt.float32
FR = mybir.dt.float32r  # single-pass PE mode (fp32 is 2 half-speed passes)
TANH = mybir.ActivationFunctionType.Tanh


def _f(ap):
    return ap.bitcast(FP)


# wpack column layout
_C_ENC = 0              # enc_w                 [128, 64]   (T0 encoder)
_C_ENCP = 64            # [0 | enc_w]           [128, 128]  (T1 encoder, zero-pad)
_C_W1 = (192, 448)      # kron(I2, W1_r)        [128, 128] per round
_C_W2 = (320, 576)      # kron(I2, W2_r)        [128, 128] per round
_C_D1 = 704             # kron(I2, dec_w1)      [128, 128]
_C_D2 = 832             # kron(I2, dec_w2)      [128, 32]
_C_BE = 864             # bias cols: enc, r0, r1, dec1 (each stacked [b; b])
NW = 868


def build_body(ctx, tc, obs_t, wpack, out, n_units):
    nc = tc.nc
    wpool = ctx.enter_context(tc.tile_pool(name="w", bufs=1))
    obs_pool = ctx.enter_context(tc.tile_pool(name="obs", bufs=8))
    h_pool = ctx.enter_context(tc.tile_pool(name="h", bufs=12))
    s_pool = ctx.enter_context(tc.tile_pool(name="s", bufs=8))
    osb_pool = ctx.enter_context(tc.tile_pool(name="osb", bufs=6))
    ps_pool = ctx.enter_context(tc.tile_pool(name="ps", bufs=3, space="PSUM"))
    po_pool = ctx.enter_context(tc.tile_pool(name="po", bufs=1, space="PSUM"))

    w = wpool.tile([D, NW], FR)
    nc.sync.dma_start(out=w[:], in_=wpack)

    w_enc = w[:, _C_ENC : _C_ENC + 64]
    w_encp = w[:, _C_ENCP : _C_ENCP + 128]
    w1 = [w[:, _C_W1[r] : _C_W1[r] + 128] for r in range(R)]
    w2 = [w[:, _C_W2[r] : _C_W2[r] + 128] for r in range(R)]
    w_d1 = w[:, _C_D1 : _C_D1 + 128]
    w_d2 = w[:, _C_D2 : _C_D2 + 32]
    b_enc = _f(w[:, _C_BE : _C_BE + 1])
    b_r = [_f(w[:, _C_BE + 1 + r : _C_BE + 2 + r]) for r in range(R)]
    b_d1 = _f(w[:, _C_BE + 3 : _C_BE + 4])

    c0 = slice(0, 512)
    c1 = slice(512, 1024)
    lo = slice(0, 64)

    # out DRAM layout is the DMA walk order itself: [u, 32 rows, 1024 tok]
    # where row = half*16 + action (host transposes back afterwards)
    out_v = out.rearrange("(u r) t -> u r t", r=32)

    for u in range(n_units):
        obs0 = obs_pool.tile([D, HALF_TOK], FR, tag="obs")
        obs1 = obs_pool.tile([D, HALF_TOK], FR, tag="obs")
        nc.sync.dma_start(out=obs0[:], in_=obs_t[u, 0])
        nc.sync.dma_start(out=obs1[:], in_=obs_t[u, 1])

        # encoder: T1 via zero-padded M=128 lhsT (start), T0 accumulates M=64
        ps_e = ps_pool.tile([128, HALF_TOK], FP, tag="ps")
        for cs in (c0, c1):
            nc.tensor.matmul(ps_e[:, cs], lhsT=w_encp, rhs=obs1[:, cs],
                             start=True, stop=False, skip_group_check=True)
            nc.tensor.matmul(ps_e[lo, cs], lhsT=w_enc, rhs=obs0[:, cs],
                             start=False, stop=True, skip_group_check=True)

        h = h_pool.tile([128, HALF_TOK], FR, tag="h")
        nc.scalar.activation(h[:], ps_e[:], TANH, bias=b_enc)

        for r in range(R):
            s = s_pool.tile([128, SAMP_HALF], FR, tag="s")
            with nc.allow_low_precision(
                reason="float32r is 4-byte fp32; PE needs f32r-typed operands"
            ):
                nc.vector.reduce_sum(
                    out=s[:],
                    in_=h.rearrange("p (g a) -> p g a", a=A),
                    axis=mybir.AxisListType.X,
                )
            ns = SAMP_HALF // 2  # samples per 512-token column block
            ps_r = ps_pool.tile([128, HALF_TOK], FP, tag="ps")
            for cs in (c0, c1):
                nc.tensor.matmul(ps_r[:, cs], lhsT=w1[r], rhs=h[:, cs],
                                 start=True, stop=False, skip_group_check=True)
            for b, cs in ((0, c0), (1, c1)):
                sb = s[:, b * ns : (b + 1) * ns].unsqueeze(2).broadcast_to(
                    [128, ns, A]
                )
                nc.tensor.matmul(ps_r[:, cs], lhsT=w2[r], rhs=sb,
                                 start=False, stop=True, skip_group_check=True)

            h = h_pool.tile([128, HALF_TOK], FR, tag="h")
            nc.scalar.activation(h[:], ps_r[:], TANH, bias=b_r[r])

        # dec1
        ps_d = ps_pool.tile([128, HALF_TOK], FP, tag="ps")
        for cs in (c0, c1):
            nc.tensor.matmul(ps_d[:, cs], lhsT=w_d1, rhs=h[:, cs],
                             skip_group_check=True)
        pre = h_pool.tile([128, HALF_TOK], FR, tag="h")
        nc.scalar.activation(pre[:], ps_d[:], TANH, bias=b_d1)

        # dec2 feature-major: logits [32, 1024]; rows 0:16 = T0 actions,
        # 16:32 = T1 actions; d2 block-diagonal weights stay loaded across
        # blocks (no per-chunk LDWEIGHTS)
        po = po_pool.tile([32, HALF_TOK], FP, tag="po")
        for cs in (c0, c1):
            nc.tensor.matmul(po[:, cs], lhsT=w_d2, rhs=pre[:, cs],
                             skip_group_check=True)

        osb = osb_pool.tile([32, HALF_TOK], FP, tag="osb")
        nc.vector.tensor_copy(osb[:], po[:])

        nc.sync.dma_start(out=out_v[u], in_=osb[:])


def build_nc(n_units=NU):
    nc = bacc.Bacc(None, target_bir_lowering=False, debug=False)
    obs_t = nc.declare_dram_parameter(
        "obs_t", [n_units, 2, D, HALF_TOK], FR, isOutput=False
    )
    wpack = nc.declare_dram_parameter("wpack", [D, NW], FR, isOutput=False)
    out = nc.declare_dram_parameter(
        "out", [n_units * 32, HALF_TOK], FP, isOutput=True
    )
    with tile.TileContext(nc) as tc:
        with ExitStack() as ctx:
            build_body(ctx, tc, obs_t[:], wpack[:], out[:], n_units)
    nc.compile()
    return nc


def fold_weights(enc_w, enc_b, comm_w, comm_b, upd_w, upd_b, dec_w1, dec_b1, dec_w2):
    """Host-side algebraic fold + packing into the wpack tensor (float64 math)."""
    f8 = np.float64
    denom = f8(max(A - 1, 1))
    wpack = np.zeros((D, NW), np.float32)

    def bd(Wm):  # kron(I2, W) for [64, x] -> [128, 2x]
        Wm = np.asarray(Wm, np.float32)
        k, m = Wm.shape
        o = np.zeros((2 * k, 2 * m), np.float32)
        o[:k, :m] = Wm
        o[k:, m:] = Wm
        return o

    wpack[:, _C_ENC : _C_ENC + 64] = np.asarray(enc_w, np.float32)
    wpack[:, _C_ENCP + 64 : _C_ENCP + 128] = np.asarray(enc_w, np.float32)
    for r in range(R):
        C = np.asarray(comm_w[r], f8)
        Ut = np.asarray(upd_w[r][:H], f8)
        Ub = np.asarray(upd_w[r][H:], f8)
        G = C @ Ub / denom
        W1 = (Ut - G).astype(np.float32)
        W2 = G.astype(np.float32)
        br = (np.asarray(comm_b[r], f8) @ Ub + np.asarray(upd_b[r], f8)).astype(
            np.float32
        )
        wpack[:, _C_W1[r] : _C_W1[r] + 128] = bd(W1)
        wpack[:, _C_W2[r] : _C_W2[r] + 128] = bd(W2)
        wpack[0:64, _C_BE + 1 + r] = br
        wpack[64:128, _C_BE + 1 + r] = br
    wpack[:, _C_D1 : _C_D1 + 128] = bd(dec_w1)
    wpack[:, _C_D2 : _C_D2 + 32] = bd(dec_w2)
    be = np.asarray(enc_b, np.float32)
    wpack[0:64, _C_BE] = be
    wpack[64:128, _C_BE] = be
    bd1 = np.asarray(dec_b1, np.float32)
    wpack[0:64, _C_BE + 3] = bd1
    wpack[64:128, _C_BE + 3] = bd1
    return wpack


def prep_obs(obs):
    """[B, A, D] -> [NCORES, NU, 2, D, HALF_TOK] feature-major contiguous."""
    obs5 = np.asarray(obs, np.float32).reshape(NCORES, NU, 2, HALF_TOK, D)
    return np.ascontiguousarray(obs5.transpose(0, 1, 2, 4, 3))


_NC_CACHE = {}


def _get_nc(n_units=NU):
    if n_units not in _NC_CACHE:
        _NC_CACHE[n_units] = build_nc(n_units)
    return _NC_CACHE[n_units]


def kernel(
    obs,
    enc_w,
    enc_b,
    comm_w,
    comm_b,
    upd_w,
    upd_b,
    dec_w1,
    dec_b1,
    dec_w2,
    dec_b2,
    _trace=False,
    _trace_kwargs=None,
):
    wpack = fold_weights(
        enc_w, enc_b, comm_w, comm_b, upd_w, upd_b, dec_w1, dec_b1, dec_w2
    )
    obs_t = prep_obs(obs)
    nc = _get_nc()
    in_maps = [{"obs_t": obs_t[i], "wpack": wpack} for i in range(NCORES)]
    res = run_bass_kernel_spmd(
        nc,
        in_maps,
        core_ids=list(range(NCORES)),
        trace=_trace,
        **(_trace_kwargs or {}),
    )
    outs = np.stack([res.results[i]["out"] for i in range(NCORES)])
    # device order is [u, half*16+e, tok]; token t = u*2048 + half*1024 + tok
    outs = outs.reshape(NCORES, NU, 2, NA, HALF_TOK)
    outs = outs.transpose(0, 1, 2, 4, 3)  # -> [core, u, half, tok, e]
    logits = outs.reshape(B, A, NA) + np.asarray(dec_b2, np.float32)[None, None, :]
    if _trace:
        return logits.astype(np.float32), res
    return logits.astype(np.float32)


# revision 16
# speedup vs baseline: 2.9535x; 1.8430x over previous
"""CommNet actor kernel for Trainium2 (Bass/Tile), 8-core data-parallel.

Math (per sample, A=32 agents, D=128 obs, H=64 hidden, NA=16 actions):
    h   = tanh(obs @ enc_w + enc_b)
    2 rounds of:  messages = h @ comm_w + comm_b
                  received = (sum_agents(messages) - messages) / (A-1)
                  h = tanh([h, received] @ upd_w + upd_b)
    out = tanh(h @ dec_w1 + dec_b1) @ dec_w2 + dec_b2

The round is folded on the host into  h' = tanh(h @ W1 + s @ W2 + b)  where
s = sum_agents(h), W1 = U_top - comm_w @ U_bot / (A-1), W2 = comm_w @ U_bot / (A-1),
b = comm_b @ U_bot + upd_b   (U_top/U_bot = upd_w[:H], upd_w[H:]).

Device layout: feature-major activations [feat, tok]. Each "unit" is 2048
tokens; the first 1024 tokens (T0) live on SBUF/PSUM partitions 0:64, the
second 1024 (T1) on partitions 64:128. All matmuls run in float32r (single-
pass PE mode; plain fp32 costs 2 half-speed passes). f32r only supports
tile_position (0,0), so the two halves are computed with block-diagonal
weights kron(I2, W) in one full-array matmul; the encoder stacks halves via
a zero-padded lhsT accumulation pair. tanh/reduce then process both halves
in single [128, 1024] instructions (full 128-lane utilization).

obs is pre-transposed on the host into the exact feature-major DMA layout, so
all HBM traffic is contiguous; the output is stored in DMA walk order and
transposed back on the host.
"""

import numpy as np
from contextlib import ExitStack

import concourse.bass as bass
import concourse.bacc as bacc
import concourse.tile as tile
from concourse import mybir
from concourse.bass_utils import run_bass_kernel_spmd

# Problem constants
B, A, D, H, NA = 16384, 32, 128, 64, 16
R = 2
NCORES = 8
S_CORE = B // NCORES          # 2048 samples per core
TOK = S_CORE * A              # 65536 tokens per core
HALF_TOK = 1024               # tokens per half-unit (32 samples)
UNIT_TOK = 2 * HALF_TOK       # 2048 tokens per unit
NU = TOK // UNIT_TOK          # 32 units per core
SAMP_HALF = HALF_TOK // A     # 32 samples per half-unit
CHUNK = 128                   # dec2 token chunk (output partition dim)
NCHUNK = HALF_TOK // CHUNK    # 8 chunks per half-unit
FP = mybir.dt.float32
FR = mybir.dt.float32r  # single-pass PE mode (fp32 is 2 half-speed passes)
TANH = mybir.ActivationFunctionType.Tanh


def _f(ap):
    return ap.bitcast(FP)


# wpack column layout
_C_ENC = 0              # enc_w                 [128, 64]   (T0 encoder)
_C_ENCP = 64            # [0 | enc_w]           [128, 128]  (T1 encoder, zero-pad)
_C_W1 = (192, 448)      # kron(I2, W1_r)        [128, 128] per round
_C_W2 = (320, 576)      # kron(I2, W2_r)        [128, 128] per round
_C_D1 = 704             # kron(I2, dec_w1)      [128, 128]
_C_D2 = 832             # kron(I2, dec_w2)      [128, 32]
_C_BE = 864             # bias cols: enc, r0, r1, dec1 (each stacked [b; b])
NW = 868


def build_body(ctx, tc, obs_t, wpack, out, n_units):
    nc = tc.nc
    wpool = ctx.enter_context(tc.tile_pool(name="w", bufs=1))
    obs_pool = ctx.enter_context(tc.tile_pool(name="obs", bufs=8))
    h_pool = ctx.enter_context(tc.tile_pool(name="h", bufs=12))
    s_pool = ctx.enter_context(tc.tile_pool(name="s", bufs=8))
    osb_pool = ctx.enter_context(tc.tile_pool(name="osb", bufs=6))
    ps_pool = ctx.enter_context(tc.tile_pool(name="ps", bufs=4, space="PSUM"))

    w = wpool.tile([D, NW], FR)
    nc.sync.dma_start(out=w[:], in_=wpack)

    w_enc = w[:, _C_ENC : _C_ENC + 64]
    w_encp = w[:, _C_ENCP : _C_ENCP + 128]
    w1 = [w[:, _C_W1[r] : _C_W1[r] + 128] for r in range(R)]
    w2 = [w[:, _C_W2[r] : _C_W2[r] + 128] for r in range(R)]
    w_d1 = w[:, _C_D1 : _C_D1 + 128]
    w_d2 = w[:, _C_D2 : _C_D2 + 32]
    b_enc = _f(w[:, _C_BE : _C_BE + 1])
    b_r = [_f(w[:, _C_BE + 1 + r : _C_BE + 2 + r]) for r in range(R)]
    b_d1 = _f(w[:, _C_BE + 3 : _C_BE + 4])

    c0 = slice(0, 512)
    c1 = slice(512, 1024)
    lo = slice(0, 64)

    # out DRAM layout is the DMA walk order itself: [u, 32 rows, 1024 tok]
    # where row = half*16 + action (host transposes back afterwards)
    out_v = out.rearrange("(u r) t -> u r t", r=32)

    def emit_loads(u):
        obs0 = obs_pool.tile([D, HALF_TOK], FR, tag="obs")
        obs1 = obs_pool.tile([D, HALF_TOK], FR, tag="obs")
        nc.sync.dma_start(out=obs0[:], in_=obs_t[u, 0])
        nc.sync.dma_start(out=obs1[:], in_=obs_t[u, 1])
        return obs0, obs1

    def emit_enc_mms(obs0, obs1):
        ps_e = ps_pool.tile([128, HALF_TOK], FP, tag="ps")
        for cs in (c0, c1):
            nc.tensor.matmul(ps_e[:, cs], lhsT=w_encp, rhs=obs1[:, cs],
                             start=True, stop=False, skip_group_check=True)
            nc.tensor.matmul(ps_e[lo, cs], lhsT=w_enc, rhs=obs0[:, cs],
                             start=False, stop=True, skip_group_check=True)
        return ps_e

    def emit_tanh(ps, bias):
        hh = h_pool.tile([128, HALF_TOK], FR, tag="h")
        nc.scalar.activation(hh[:], ps[:], TANH, bias=bias)
        return hh

    def emit_reduce(hh):
        s = s_pool.tile([128, SAMP_HALF], FR, tag="s")
        with nc.allow_low_precision(
            reason="float32r is 4-byte fp32; PE needs f32r-typed operands"
        ):
            nc.vector.reduce_sum(
                out=s[:],
                in_=hh.rearrange("p (g a) -> p g a", a=A),
                axis=mybir.AxisListType.X,
            )
        return s

    def emit_round_mms(r, hh, s):
        ns = SAMP_HALF // 2  # samples per 512-token column block
        ps_r = ps_pool.tile([128, HALF_TOK], FP, tag="ps")
        for cs in (c0, c1):
            nc.tensor.matmul(ps_r[:, cs], lhsT=w1[r], rhs=hh[:, cs],
                             start=True, stop=False, skip_group_check=True)
        for b, cs in ((0, c0), (1, c1)):
            sb = s[:, b * ns : (b + 1) * ns].unsqueeze(2).broadcast_to(
                [128, ns, A]
            )
            nc.tensor.matmul(ps_r[:, cs], lhsT=w2[r], rhs=sb,
                             start=False, stop=True, skip_group_check=True)
        return ps_r

    def emit_dec1_mms(hh):
        ps_d = ps_pool.tile([128, HALF_TOK], FP, tag="ps")
        for cs in (c0, c1):
            nc.tensor.matmul(ps_d[:, cs], lhsT=w_d1, rhs=hh[:, cs],
                             skip_group_check=True)
        return ps_d

    def emit_dec2(u, pre):
        # dec2 feature-major: logits [32, 1024] in the top 32 partitions of a
        # shared-pool psum tile; rows 0:16 = T0 actions, 16:32 = T1
        po = ps_pool.tile([128, HALF_TOK], FP, tag="ps")
        for cs in (c0, c1):
            nc.tensor.matmul(po[0:32, cs], lhsT=w_d2, rhs=pre[:, cs],
                             skip_group_check=True)
        osb = osb_pool.tile([32, HALF_TOK], FP, tag="osb")
        nc.vector.tensor_copy(osb[:], po[0:32, :])
        nc.sync.dma_start(out=out_v[u], in_=osb[:])

    # Two units are emitted in lockstep (stage-interleaved) so one unit's
    # matmuls fill the PE gaps left by the other unit's tanh/reduce stages —
    # without this the PE idles ~1.3us at every stage boundary and the HAM
    # clock-gate re-throttles it to 1.2 GHz.
    assert n_units % 2 == 0
    for pair in range(n_units // 2):
        uA, uB = 2 * pair, 2 * pair + 1
        obsA = emit_loads(uA)
        obsB = emit_loads(uB)
        psA = emit_enc_mms(*obsA)
        psB = emit_enc_mms(*obsB)
        hA = emit_tanh(psA, b_enc)
        hB = emit_tanh(psB, b_enc)
        for r in range(R):
            sA = emit_reduce(hA)
            sB = emit_reduce(hB)
            psA = emit_round_mms(r, hA, sA)
            psB = emit_round_mms(r, hB, sB)
            hA = emit_tanh(psA, b_r[r])
            hB = emit_tanh(psB, b_r[r])
        psA = emit_dec1_mms(hA)
        psB = emit_dec1_mms(hB)
        preA = emit_tanh(psA, b_d1)
        preB = emit_tanh(psB, b_d1)
        emit_dec2(uA, preA)
        emit_dec2(uB, preB)

def build_nc(n_units=NU):
    nc = bacc.Bacc(None, target_bir_lowering=False, debug=False)
    obs_t = nc.declare_dram_parameter(
        "obs_t", [n_units, 2, D, HALF_TOK], FR, isOutput=False
    )
    wpack = nc.declare_dram_parameter("wpack", [D, NW], FR, isOutput=False)
    out = nc.declare_dram_parameter(
        "out", [n_units * 32, HALF_TOK], FP, isOutput=True
    )
    with tile.TileContext(nc) as tc:
        with ExitStack() as ctx:
            build_body(ctx, tc, obs_t[:], wpack[:], out[:], n_units)
    nc.compile()
    return nc


def fold_weights(enc_w, enc_b, comm_w, comm_b, upd_w, upd_b, dec_w1, dec_b1, dec_w2):
    """Host-side algebraic fold + packing into the wpack tensor (float64 math)."""
    f8 = np.float64
    denom = f8(max(A - 1, 1))
    wpack = np.zeros((D, NW), np.float32)

    def bd(Wm):  # kron(I2, W) for [64, x] -> [128, 2x]
        Wm = np.asarray(Wm, np.float32)
        k, m = Wm.shape
        o = np.zeros((2 * k, 2 * m), np.float32)
        o[:k, :m] = Wm
        o[k:, m:] = Wm
        return o

    wpack[:, _C_ENC : _C_ENC + 64] = np.asarray(enc_w, np.float32)
    wpack[:, _C_ENCP + 64 : _C_ENCP + 128] = np.asarray(enc_w, np.float32)
    for r in range(R):
        C = np.asarray(comm_w[r], f8)
        Ut = np.asarray(upd_w[r][:H], f8)
        Ub = np.asarray(upd_w[r][H:], f8)
        G = C @ Ub / denom
        W1 = (Ut - G).astype(np.float32)
        W2 = G.astype(np.float32)
        br = (np.asarray(comm_b[r], f8) @ Ub + np.asarray(upd_b[r], f8)).astype(
            np.float32
        )
        wpack[:, _C_W1[r] : _C_W1[r] + 128] = bd(W1)
        wpack[:, _C_W2[r] : _C_W2[r] + 128] = bd(W2)
        wpack[0:64, _C_BE + 1 + r] = br
        wpack[64:128, _C_BE + 1 + r] = br
    wpack[:, _C_D1 : _C_D1 + 128] = bd(dec_w1)
    wpack[:, _C_D2 : _C_D2 + 32] = bd(dec_w2)
    be = np.asarray(enc_b, np.float32)
    wpack[0:64, _C_BE] = be
    wpack[64:128, _C_BE] = be
    bd1 = np.asarray(dec_b1, np.float32)
    wpack[0:64, _C_BE + 3] = bd1
    wpack[64:128, _C_BE + 3] = bd1
    return wpack


def prep_obs(obs):
    """[B, A, D] -> [NCORES, NU, 2, D, HALF_TOK] feature-major contiguous."""
    obs5 = np.asarray(obs, np.float32).reshape(NCORES, NU, 2, HALF_TOK, D)
    return np.ascontiguousarray(obs5.transpose(0, 1, 2, 4, 3))


_NC_CACHE = {}


def _get_nc(n_units=NU):
    if n_units not in _NC_CACHE:
        _NC_CACHE[n_units] = build_nc(n_units)
    return _NC_CACHE[n_units]


def kernel(
    obs,
    enc_w,
    enc_b,
    comm_w,
    comm_b,
    upd_w,
    upd_b,
    dec_w1,
    dec_b1,
    dec_w2,
    dec_b2,
    _trace=False,
    _trace_kwargs=None,
):
    wpack = fold_weights(
        enc_w, enc_b, comm_w, comm_b, upd_w, upd_b, dec_w1, dec_b1, dec_w2
    )
    obs_t = prep_obs(obs)
    nc = _get_nc()
    in_maps = [{"obs_t": obs_t[i], "wpack": wpack} for i in range(NCORES)]
    res = run_bass_kernel_spmd(
        nc,
        in_maps,
        core_ids=list(range(NCORES)),
        trace=_trace,
        **(_trace_kwargs or {}),
    )
    outs = np.stack([res.results[i]["out"] for i in range(NCORES)])
    # device order is [u, half*16+e, tok]; token t = u*2048 + half*1024 + tok
    outs = outs.reshape(NCORES, NU, 2, NA, HALF_TOK)
    outs = outs.transpose(0, 1, 2, 4, 3)  # -> [core, u, half, tok, e]
    logits = outs.reshape(B, A, NA) + np.asarray(dec_b2, np.float32)[None, None, :]
    if _trace:
        return logits.astype(np.float32), res
    return logits.astype(np.float32)
